# revision 4
# baseline (speedup 1.0000x reference)
"""Tensor-parallel multi-head attention (RoPE) kernel for 8 Trainium2 cores.

Shapes (hardcoded): x [2, 2048, 1024], 16 heads x head_dim 64.
Sharding: core c -> batch b = c//4, head-group hg = c%4 (4 heads = 256
projection columns). Each core computes q/k/v projections for its head
columns, RoPE, attention, and a partial out-projection over its 256 rows
of o_w; the host sums the 4 partials per batch and adds o_b.

Device-side layout choices:
  - qT/kT stored transposed [head_dim-major on partitions, tokens on free]
    so QK^T contracts over partitions directly.
  - scores computed transposed S^T[k, q]; softmax max-subtraction is
    skipped (scores are O(+-6), fp32 exp is exact enough).
  - PV uses stationary [V | 1] so one accumulation produces both the
    unnormalized output and the softmax denominators (row 64).
"""

import sys
import numpy as np

for p in ("/opt/trn_rl_repo", "/root/.axon_site/_ro/trn_rl_repo"):
    if p not in sys.path:
        sys.path.insert(0, p)

B, L, D = 2, 2048, 1024
H, HD = 16, 64
NCORES = 8
HG = 4                  # head-groups == cores per batch
EL = D // HG            # 256 projection columns per core
ET = EL // 128          # 2 e-tiles
DT = D // 128           # 8 d-tiles
TT = L // 128           # 16 token tiles
NH = H // HG            # 4 heads per core

_cache = {}


def _build():
    import concourse.mybir as mybir
    from concourse import bacc, tile

    F32 = mybir.dt.float32
    AF = mybir.ActivationFunctionType

    nc = bacc.Bacc("TRN2", target_bir_lowering=False, debug=False,
                   num_devices=NCORES)

    xT = nc.dram_tensor("xT", [D, L], F32, kind="ExternalInput").ap()
    wq = nc.dram_tensor("wq", [D, EL], F32, kind="ExternalInput").ap()
    wk = nc.dram_tensor("wk", [D, EL], F32, kind="ExternalInput").ap()
    wv = nc.dram_tensor("wv", [D, EL], F32, kind="ExternalInput").ap()
    wo = nc.dram_tensor("wo", [EL, D], F32, kind="ExternalInput").ap()
    bq = nc.dram_tensor("bq", [ET, 128, 1], F32, kind="ExternalInput").ap()
    bk = nc.dram_tensor("bk", [ET, 128, 1], F32, kind="ExternalInput").ap()
    bv = nc.dram_tensor("bv", [ET, 128, 1], F32, kind="ExternalInput").ap()
    cosb = nc.dram_tensor("cosb", [128, L], F32, kind="ExternalInput").ap()
    sinb = nc.dram_tensor("sinb", [128, L], F32, kind="ExternalInput").ap()
    outT = nc.dram_tensor("outT", [D, L], F32, kind="ExternalOutput").ap()

    with tile.TileContext(nc) as tc:
        with tc.tile_pool(name="persist", bufs=1) as P:
            qT = [P.tile([128, L], F32, name=f"qT{e}") for e in range(ET)]
            kT = [P.tile([128, L], F32, name=f"kT{e}") for e in range(ET)]
            Vsb = [P.tile([128, NH * 65], F32, name=f"V{t}") for t in range(TT)]
            ao = [P.tile([128, L], F32, name=f"ao{e}") for e in range(ET)]
            ones1 = P.tile([1, 64], F32)
            nc.vector.memset(ones1[:], 1.0)
            bvt = [P.tile([128, 1], F32, name=f"bv{e}") for e in range(ET)]
            for e in range(ET):
                nc.sync.dma_start(bvt[e][:], bv[e])

            # ---------------- Phase B: q/k/v projections + RoPE ----------
            with (
                tc.tile_pool(name="xw", bufs=1) as XW,
                tc.tile_pool(name="pb", bufs=2, space="PSUM") as PB,
                tc.tile_pool(name="ropet", bufs=2) as RT,
            ):
                xts = [XW.tile([128, L], F32, name=f"x{d}") for d in range(DT)]
                for d in range(DT):
                    nc.sync.dma_start(xts[d][:], xT[d * 128:(d + 1) * 128, :])
                wqs = [XW.tile([128, EL], F32, name=f"wq{d}") for d in range(DT)]
                wks = [XW.tile([128, EL], F32, name=f"wk{d}") for d in range(DT)]
                wvs = [XW.tile([128, EL], F32, name=f"wv{d}") for d in range(DT)]
                for d in range(DT):
                    nc.sync.dma_start(wqs[d][:], wq[d * 128:(d + 1) * 128, :])
                    nc.sync.dma_start(wks[d][:], wk[d * 128:(d + 1) * 128, :])
                    nc.sync.dma_start(wvs[d][:], wv[d * 128:(d + 1) * 128, :])
                cosbt = XW.tile([128, L], F32)
                sinbt = XW.tile([128, L], F32)
                nc.sync.dma_start(cosbt[:], cosb[:])
                nc.sync.dma_start(sinbt[:], sinb[:])
                bqt = [XW.tile([128, 1], F32, name=f"bq{e}") for e in range(ET)]
                bkt = [XW.tile([128, 1], F32, name=f"bk{e}") for e in range(ET)]
                for e in range(ET):
                    nc.sync.dma_start(bqt[e][:], bq[e])
                    nc.sync.dma_start(bkt[e][:], bk[e])

                for wts, bts, dst in ((wqs, bqt, qT), (wks, bkt, kT)):
                    for e in range(ET):
                        ps = PB.tile([128, L], F32, tag="ps")
                        for d in range(DT):
                            for c in range(0, L, 512):
                                nc.tensor.matmul(
                                    ps[:, c:c + 512],
                                    wts[d][:, e * 128:(e + 1) * 128],
                                    xts[d][:, c:c + 512],
                                    start=(d == 0), stop=(d == DT - 1),
                                    skip_group_check=True)
                        nc.scalar.activation(dst[e][:], ps[:], AF.Identity,
                                             bias=bts[e][:])
                        # RoPE: build rotate_half source via partition-shifted
                        # SBUF->SBUF DMA, then 2 muls + add.
                        rs = RT.tile([128, L], F32, tag="rs")
                        tmp = RT.tile([128, L], F32, tag="tmp")
                        for g in range(4):
                            s0 = g * 32
                            d0 = s0 + 32 if g % 2 == 0 else s0 - 32
                            nc.sync.dma_start(rs[s0:s0 + 32, :],
                                              dst[e][d0:d0 + 32, :])
                        nc.vector.tensor_mul(tmp[:], dst[e][:], cosbt[:])
                        nc.vector.tensor_mul(rs[:], rs[:], sinbt[:])
                        nc.vector.tensor_add(dst[e][:], tmp[:], rs[:])

                for t in range(TT):
                    ps = PB.tile([128, EL], F32, tag="ps")
                    for d in range(DT):
                        nc.tensor.matmul(
                            ps[:], xts[d][:, t * 128:(t + 1) * 128], wvs[d][:],
                            start=(d == 0), stop=(d == DT - 1),
                            skip_group_check=True)
                    dv = Vsb[t][:].rearrange("p (h c) -> p h c", c=65)
                    nc.scalar.activation(
                        dv[:, :, 0:64],
                        ps[:].rearrange("p (h c) -> p h c", c=64),
                        AF.Identity)
                    nc.vector.memset(dv[:, :, 64:65], 1.0)

            # ---------------- Phase C: attention per head ----------------
            with (
                tc.tile_pool(name="po", bufs=1, space="PSUM") as PO,
                tc.tile_pool(name="pscr", bufs=2, space="PSUM") as PS2,
                tc.tile_pool(name="esb", bufs=3) as EP,
                tc.tile_pool(name="smallsb", bufs=2) as SS,
            ):
                for h in range(NH):
                    e, off = divmod(h, 2)
                    off *= 64
                    qh = qT[e][off:off + 64, :]
                    kh = kT[e][off:off + 64, :]
                    op = PO.tile([65, L], F32, tag="op")
                    for tk in range(TT):
                        for c0 in range(0, L, 1024):
                            sp = PS2.tile([128, 1024], F32, tag="scr")
                            for c in range(0, 1024, 512):
                                nc.tensor.matmul(
                                    sp[:, c:c + 512],
                                    kh[:, tk * 128:(tk + 1) * 128],
                                    qh[:, c0 + c:c0 + c + 512],
                                    start=True, stop=True,
                                    skip_group_check=True)
                            eb = EP.tile([128, 1024], F32, tag="eb")
                            nc.scalar.activation(eb[:], sp[:], AF.Exp,
                                                 scale=0.125)
                            for c in range(0, 1024, 512):
                                nc.tensor.matmul(
                                    op[:, c0 + c:c0 + c + 512],
                                    Vsb[tk][:, h * 65:h * 65 + 65],
                                    eb[:, c:c + 512],
                                    start=(tk == 0), stop=(tk == TT - 1),
                                    skip_group_check=True)
                    rb = SS.tile([1, L], F32, tag="rb")
                    nc.vector.reciprocal(rb[:], op[64:65, :])
                    for c0 in range(0, L, 1024):
                        rp = PS2.tile([64, 1024], F32, tag="scr")
                        for c in range(0, 1024, 512):
                            nc.tensor.matmul(
                                rp[:, c:c + 512], ones1[:],
                                rb[:, c0 + c:c0 + c + 512],
                                start=True, stop=True, skip_group_check=True)
                        rbb = EP.tile([64, 1024], F32, tag="eb")
                        nc.scalar.activation(rbb[:], rp[:], AF.Identity)
                        nc.vector.tensor_mul(ao[e][off:off + 64, c0:c0 + 1024],
                                             op[0:64, c0:c0 + 1024], rbb[:])
                    nc.scalar.activation(ao[e][off:off + 64, :],
                                         ao[e][off:off + 64, :],
                                         AF.Identity, bias=bvt[e][off:off + 64, :])

            # ---------------- Phase D: partial out-projection ------------
            with (
                tc.tile_pool(name="wod", bufs=1) as WOP,
                tc.tile_pool(name="od", bufs=2) as OD,
                tc.tile_pool(name="pd", bufs=2, space="PSUM") as PD,
            ):
                wos = [WOP.tile([128, D], F32, name=f"wo{e}") for e in range(ET)]
                for e in range(ET):
                    nc.sync.dma_start(wos[e][:], wo[e * 128:(e + 1) * 128, :])
                for dc in range(DT):
                    pdt = PD.tile([128, L], F32, tag="pd")
                    for e in range(ET):
                        for c in range(0, L, 512):
                            nc.tensor.matmul(
                                pdt[:, c:c + 512],
                                wos[e][:, dc * 128:(dc + 1) * 128],
                                ao[e][:, c:c + 512],
                                start=(e == 0), stop=(e == ET - 1),
                                skip_group_check=True)
                    osb = OD.tile([128, L], F32, tag="osb")
                    nc.vector.tensor_copy(osb[:], pdt[:])
                    nc.sync.dma_start(outT[dc * 128:(dc + 1) * 128, :], osb[:])

    nc.compile()
    return nc


def _rope_tables():
    inv = 1.0 / (10000.0 ** (np.arange(0, HD, 2, dtype=np.float32) / HD))
    t = np.arange(L, dtype=np.float32)
    fr = t[:, None] * inv[None, :]                    # [L, 32]
    emb = np.concatenate([fr, fr], axis=1)            # [L, 64]
    cos, sin = np.cos(emb), np.sin(emb)               # [L, 64]
    # device layout [128, L]: row p covers head-dim i = p % 64, two heads
    # stacked per 128-partition tile; sin carries the rotate_half sign.
    i = np.arange(128) % HD
    cosb = cos.T[i, :]                                # [128, L]
    sg = np.where(i < HD // 2, -1.0, 1.0).astype(np.float32)
    sinb = sin.T[i, :] * sg[:, None]
    return np.ascontiguousarray(cosb, np.float32), \
        np.ascontiguousarray(sinb, np.float32)


def kernel(x, q_w, q_b, k_w, k_b, v_w, v_b, o_w, o_b):
    from concourse.bass_utils import run_bass_kernel_spmd

    x = np.asarray(x, np.float32)
    assert x.shape == (B, L, D), x.shape

    if "nc" not in _cache:
        _cache["nc"] = _build()
    nc = _cache["nc"]

    cosb, sinb = _rope_tables()
    qwT = np.ascontiguousarray(np.asarray(q_w, np.float32).T)  # [D, D] eff
    kwT = np.ascontiguousarray(np.asarray(k_w, np.float32).T)
    vwT = np.ascontiguousarray(np.asarray(v_w, np.float32).T)
    owT = np.ascontiguousarray(np.asarray(o_w, np.float32).T)
    xTb = [np.ascontiguousarray(x[b].T) for b in range(B)]

    in_maps = []
    for c in range(NCORES):
        b, hg = divmod(c, HG)
        er = slice(hg * EL, (hg + 1) * EL)
        in_maps.append({
            "xT": xTb[b],
            "wq": np.ascontiguousarray(qwT[:, er]),
            "wk": np.ascontiguousarray(kwT[:, er]),
            "wv": np.ascontiguousarray(vwT[:, er]),
            "wo": np.ascontiguousarray(owT[er, :]),
            "bq": np.ascontiguousarray(
                np.asarray(q_b, np.float32)[er].reshape(ET, 128, 1)),
            "bk": np.ascontiguousarray(
                np.asarray(k_b, np.float32)[er].reshape(ET, 128, 1)),
            "bv": np.ascontiguousarray(
                np.asarray(v_b, np.float32)[er].reshape(ET, 128, 1)),
            "cosb": cosb,
            "sinb": sinb,
        })

    res = run_bass_kernel_spmd(nc, in_maps, list(range(NCORES)))

    out = np.zeros((B, L, D), np.float32)
    for c in range(NCORES):
        b = c // HG
        out[b] += res.results[c]["outT"].T
    out += np.asarray(o_b, np.float32)[None, None, :]
    return out


# revision 9
# speedup vs baseline: 1.6050x; 1.6050x over previous
"""Tensor-parallel multi-head attention (RoPE) kernel for 8 Trainium2 cores.

Shapes (hardcoded): x [2, 2048, 1024], 16 heads x head_dim 64.
Sharding: core c -> batch b = c//4, head-group hg = c%4 (4 heads = 256
projection columns). Each core computes q/k/v projections for its head
columns, RoPE, attention, and a partial out-projection over its 256 rows
of o_w; the host sums the 4 partials per batch and adds o_b (plus the
v_b @ o_w term, which passes through attention linearly).

Device-side layout choices:
  - qT/kT stored transposed [head_dim on partitions, tokens on free]
    so QK^T contracts over partitions directly.
  - scores computed transposed S^T[k, q]; softmax max-subtraction is
    skipped (scores are O(+-6), fp32 exp is exact enough).
  - PV uses stationary [V | 1] so one accumulation produces both the
    unnormalized output and the softmax denominators (row 64).
  - matmuls run as float32r (TF32-like single-pass, 4x faster than
    fp32's two half-rate passes); the softmax-normalizer broadcast
    matmuls stay plain fp32.
  - scalar engine does only `exp`; every copy/bias lands on the vector
    engine so ACT and PE stay the co-bottleneck pair.
"""

import sys
import numpy as np

for p in ("/opt/trn_rl_repo", "/root/.axon_site/_ro/trn_rl_repo"):
    if p not in sys.path:
        sys.path.insert(0, p)

B, L, D = 2, 2048, 1024
H, HD = 16, 64
NCORES = 8
HG = 4                  # head-groups == cores per batch
EL = D // HG            # 256 projection columns per core
ET = EL // 128          # 2 e-tiles
DT = D // 128           # 8 d-tiles
TT = L // 128           # 16 token tiles
NH = H // HG            # 4 heads per core

_cache = {}


def _build():
    import concourse.mybir as mybir
    from concourse import bacc, tile

    F32 = mybir.dt.float32
    F32R = mybir.dt.float32r
    AF = mybir.ActivationFunctionType

    nc = bacc.Bacc("TRN2", target_bir_lowering=False, debug=False,
                   num_devices=NCORES)

    xT = nc.dram_tensor("xT", [D, L], F32R, kind="ExternalInput").ap()
    wq = nc.dram_tensor("wq", [D, EL], F32R, kind="ExternalInput").ap()
    wk = nc.dram_tensor("wk", [D, EL], F32R, kind="ExternalInput").ap()
    wv = nc.dram_tensor("wv", [D, EL], F32R, kind="ExternalInput").ap()
    wo = nc.dram_tensor("wo", [EL, D], F32R, kind="ExternalInput").ap()
    bq = nc.dram_tensor("bq", [ET, 128, 1], F32, kind="ExternalInput").ap()
    bk = nc.dram_tensor("bk", [ET, 128, 1], F32, kind="ExternalInput").ap()
    cosb = nc.dram_tensor("cosb", [128, L], F32, kind="ExternalInput").ap()
    onesd = nc.dram_tensor("onesd", [128, NH, 1], F32R, kind="ExternalInput").ap()
    sinb = nc.dram_tensor("sinb", [128, L], F32, kind="ExternalInput").ap()
    outT = nc.dram_tensor("outT", [D, L], F32, kind="ExternalOutput").ap()

    with tile.TileContext(nc) as tc:
        with tc.tile_pool(name="persist", bufs=1) as P:
            qT = [P.tile([128, L], F32R, name=f"qT{e}") for e in range(ET)]
            kT = [P.tile([128, L], F32R, name=f"kT{e}") for e in range(ET)]
            Vsb = [P.tile([128, NH * 65], F32R, name=f"V{t}") for t in range(TT)]
            ao = [P.tile([128, L], F32R, name=f"ao{e}") for e in range(ET)]
            ones1 = P.tile([1, 64], F32)
            nc.vector.memset(ones1[:], 1.0)

            # ---------------- Phase B: q/k/v projections + RoPE ----------
            with (
                tc.tile_pool(name="xw", bufs=1) as XW,
                tc.tile_pool(name="pb", bufs=2, space="PSUM") as PB,
                tc.tile_pool(name="ropet", bufs=2) as RT,
            ):
                xts = [XW.tile([128, L], F32R, name=f"x{d}") for d in range(DT)]
                wqs = [XW.tile([128, EL], F32R, name=f"wq{d}") for d in range(DT)]
                wks = [XW.tile([128, EL], F32R, name=f"wk{d}") for d in range(DT)]
                wvs = [XW.tile([128, EL], F32R, name=f"wv{d}") for d in range(DT)]
                for d in range(DT):
                    nc.sync.dma_start(wqs[d][:], wq[d * 128:(d + 1) * 128, :])
                    nc.sync.dma_start(xts[d][:], xT[d * 128:(d + 1) * 128, :])
                    nc.sync.dma_start(wks[d][:], wk[d * 128:(d + 1) * 128, :])
                    nc.sync.dma_start(wvs[d][:], wv[d * 128:(d + 1) * 128, :])
                cosbt = XW.tile([128, L], F32)
                sinbt = XW.tile([128, L], F32)
                nc.sync.dma_start(cosbt[:], cosb[:])
                nc.sync.dma_start(sinbt[:], sinb[:])
                bqt = [XW.tile([128, 1], F32, name=f"bq{e}") for e in range(ET)]
                bkt = [XW.tile([128, 1], F32, name=f"bk{e}") for e in range(ET)]
                for e in range(ET):
                    nc.sync.dma_start(bqt[e][:], bq[e])
                    nc.sync.dma_start(bkt[e][:], bk[e])

                for wts, bts, dst in ((wqs, bqt, qT), (wks, bkt, kT)):
                    for e in range(ET):
                        ps = PB.tile([128, L], F32, tag="ps")
                        for d in range(DT):
                            for c in range(0, L, 512):
                                nc.tensor.matmul(
                                    ps[:, c:c + 512],
                                    wts[d][:, e * 128:(e + 1) * 128],
                                    xts[d][:, c:c + 512],
                                    start=(d == 0), stop=(d == DT - 1),
                                    skip_group_check=True)
                        nc.vector.tensor_scalar_add(dst[e][:], ps[:], bts[e][:])
                        # RoPE: build rotate_half source via partition-shifted
                        # SBUF->SBUF DMA, then 2 muls + add.
                        rs = RT.tile([128, L], F32, tag="rs")
                        tmp = RT.tile([128, L], F32, tag="tmp")
                        for g in range(4):
                            s0 = g * 32
                            d0 = s0 + 32 if g % 2 == 0 else s0 - 32
                            nc.sync.dma_start(rs[s0:s0 + 32, :],
                                              dst[e][d0:d0 + 32, :].bitcast(F32))
                        nc.vector.tensor_mul(tmp[:], dst[e][:], cosbt[:])
                        nc.vector.tensor_mul(rs[:], rs[:], sinbt[:])
                        nc.vector.tensor_add(dst[e][:], tmp[:], rs[:])

                for t in range(TT):
                    ps = PB.tile([128, EL], F32, tag="ps")
                    for d in range(DT):
                        nc.tensor.matmul(
                            ps[:], xts[d][:, t * 128:(t + 1) * 128],
                            wvs[d][:],
                            start=(d == 0), stop=(d == DT - 1),
                            skip_group_check=True)
                    dv = Vsb[t][:].rearrange("p (h c) -> p h c", c=65)
                    nc.vector.tensor_copy(
                        dv[:, :, 0:64],
                        ps[:].rearrange("p (h c) -> p h c", c=64))
                    nc.sync.dma_start(dv[:, :, 64:65], onesd[:])

            # ---------------- Phase C: attention per head ----------------
            with (
                tc.tile_pool(name="po", bufs=1, space="PSUM") as PO,
                tc.tile_pool(name="pscr", bufs=2, space="PSUM") as PS2,
                tc.tile_pool(name="esb", bufs=3) as EP,
                tc.tile_pool(name="smallsb", bufs=2) as SS,
            ):
                for h in range(NH):
                    e, off = divmod(h, 2)
                    off *= 64
                    qh = qT[e][off:off + 64, :]
                    kh = kT[e][off:off + 64, :]
                    op = PO.tile([65, L], F32, tag="op")
                    for tk in range(TT):
                        for c0 in range(0, L, 1024):
                            sp = PS2.tile([128, 1024], F32, tag="scr")
                            for c in range(0, 1024, 512):
                                nc.tensor.matmul(
                                    sp[:, c:c + 512],
                                    kh[:, tk * 128:(tk + 1) * 128],
                                    qh[:, c0 + c:c0 + c + 512],
                                    start=True, stop=True,
                                    skip_group_check=True)
                            eb = EP.tile([128, 1024], F32R, tag="eb")
                            nc.scalar.activation(eb[:], sp[:], AF.Exp,
                                                 scale=0.125)
                            for c in range(0, 1024, 512):
                                nc.tensor.matmul(
                                    op[:, c0 + c:c0 + c + 512],
                                    Vsb[tk][:, h * 65:h * 65 + 65],
                                    eb[:, c:c + 512],
                                    start=(tk == 0), stop=(tk == TT - 1),
                                    skip_group_check=True)
                    # Move raw output+denominators to SBUF immediately so the
                    # PSUM accumulator frees for the next head while the
                    # normalize (slow 1-lane reciprocal) runs on DVE.
                    oraw = SS.tile([65, L], F32, tag="oraw")
                    nc.vector.tensor_copy(oraw[:], op[:])
                    rb = SS.tile([1, L], F32, tag="rb")
                    nc.vector.reciprocal(rb[:], oraw[64:65, :])
                    for c0 in range(0, L, 1024):
                        rp = PS2.tile([64, 1024], F32, tag="scr")
                        for c in range(0, 1024, 512):
                            nc.tensor.matmul(
                                rp[:, c:c + 512], ones1[:],
                                rb[:, c0 + c:c0 + c + 512],
                                start=True, stop=True, skip_group_check=True)
                        nc.vector.tensor_mul(ao[e][off:off + 64, c0:c0 + 1024],
                                             oraw[0:64, c0:c0 + 1024], rp[:])

            # ---------------- Phase D: partial out-projection ------------
            with (
                tc.tile_pool(name="wod", bufs=1) as WOP,
                tc.tile_pool(name="od", bufs=2) as OD,
                tc.tile_pool(name="pd", bufs=2, space="PSUM") as PD,
            ):
                wos = [WOP.tile([128, D], F32R, name=f"wo{e}") for e in range(ET)]
                for e in range(ET):
                    nc.sync.dma_start(wos[e][:], wo[e * 128:(e + 1) * 128, :])
                for dc in range(DT):
                    pdt = PD.tile([128, L], F32, tag="pd")
                    for e in range(ET):
                        for c in range(0, L, 512):
                            nc.tensor.matmul(
                                pdt[:, c:c + 512],
                                wos[e][:, dc * 128:(dc + 1) * 128],
                                ao[e][:, c:c + 512],
                                start=(e == 0), stop=(e == ET - 1),
                                skip_group_check=True)
                    osb = OD.tile([128, L], F32, tag="osb")
                    nc.vector.tensor_copy(osb[:], pdt[:])
                    nc.sync.dma_start(outT[dc * 128:(dc + 1) * 128, :], osb[:])

    nc.compile()
    return nc


def _rope_tables():
    inv = 1.0 / (10000.0 ** (np.arange(0, HD, 2, dtype=np.float32) / HD))
    t = np.arange(L, dtype=np.float32)
    fr = t[:, None] * inv[None, :]                    # [L, 32]
    emb = np.concatenate([fr, fr], axis=1)            # [L, 64]
    cos, sin = np.cos(emb), np.sin(emb)               # [L, 64]
    # device layout [128, L]: row p covers head-dim i = p % 64, two heads
    # stacked per 128-partition tile; sin carries the rotate_half sign.
    i = np.arange(128) % HD
    cosb = cos.T[i, :]                                # [128, L]
    sg = np.where(i < HD // 2, -1.0, 1.0).astype(np.float32)
    sinb = sin.T[i, :] * sg[:, None]
    return np.ascontiguousarray(cosb, np.float32), \
        np.ascontiguousarray(sinb, np.float32)


def _in_maps(x, q_w, q_b, k_w, k_b, v_w, o_w):
    cosb, sinb = _rope_tables()
    qwT = np.ascontiguousarray(np.asarray(q_w, np.float32).T)  # [D, D] eff
    kwT = np.ascontiguousarray(np.asarray(k_w, np.float32).T)
    vwT = np.ascontiguousarray(np.asarray(v_w, np.float32).T)
    owT = np.ascontiguousarray(np.asarray(o_w, np.float32).T)
    xTb = [np.ascontiguousarray(x[b].T) for b in range(B)]
    maps = []
    for c in range(NCORES):
        b, hg = divmod(c, HG)
        er = slice(hg * EL, (hg + 1) * EL)
        maps.append({
            "xT": xTb[b],
            "wq": np.ascontiguousarray(qwT[:, er]),
            "wk": np.ascontiguousarray(kwT[:, er]),
            "wv": np.ascontiguousarray(vwT[:, er]),
            "wo": np.ascontiguousarray(owT[er, :]),
            "bq": np.ascontiguousarray(
                np.asarray(q_b, np.float32)[er].reshape(ET, 128, 1)),
            "bk": np.ascontiguousarray(
                np.asarray(k_b, np.float32)[er].reshape(ET, 128, 1)),
            "cosb": cosb,
            "sinb": sinb,
            "onesd": np.ones((128, NH, 1), np.float32),
        })
    return maps


def kernel(x, q_w, q_b, k_w, k_b, v_w, v_b, o_w, o_b):
    from concourse.bass_utils import run_bass_kernel_spmd

    x = np.asarray(x, np.float32)
    assert x.shape == (B, L, D), x.shape

    if "nc" not in _cache:
        _cache["nc"] = _build()
    nc = _cache["nc"]

    in_maps = _in_maps(x, q_w, q_b, k_w, k_b, v_w, o_w)
    res = run_bass_kernel_spmd(nc, in_maps, list(range(NCORES)))

    out = np.zeros((B, L, D), np.float32)
    for c in range(NCORES):
        b = c // HG
        out[b] += res.results[c]["outT"].T
    # o_b, plus v_b's contribution (v_b flows through softmax-weighted
    # averaging unchanged, then through the out-projection).
    extra = np.asarray(o_b, np.float32) + \
        np.asarray(v_b, np.float32) @ np.asarray(o_w, np.float32).T
    out += extra[None, None, :]
    return out


# revision 15
# speedup vs baseline: 2.3508x; 1.4647x over previous
"""Tensor-parallel multi-head attention (RoPE) kernel for 8 Trainium2 cores.

Shapes (hardcoded): x [2, 2048, 1024], 16 heads x head_dim 64.
Sharding: core c -> batch b = c//4, head-group hg = c%4 (4 heads = 256
projection columns). Each core computes q/k/v projections for its head
columns, RoPE, attention, and a partial out-projection over its 256 rows
of o_w; the host sums the 4 partials per batch and adds o_b (plus the
v_b @ o_w term, which passes through attention linearly).

Device-side layout choices:
  - qT/kT stored transposed [head_dim on partitions, tokens on free]
    so QK^T contracts over partitions directly.
  - scores computed transposed S^T[k, q]; softmax max-subtraction is
    skipped (scores are O(+-6), fp32 exp is exact enough).
  - PV uses stationary [V | 1] so one accumulation produces both the
    unnormalized output and the softmax denominators (row 64).
  - matmuls run as float32r (TF32-like single-pass, 4x faster than
    fp32's two half-rate passes); the softmax-normalizer broadcast
    matmuls stay plain fp32.
  - scalar engine does only `exp`; every copy/bias lands on the vector
    engine so ACT and PE stay the co-bottleneck pair.
"""

import sys
import numpy as np

for p in ("/opt/trn_rl_repo", "/root/.axon_site/_ro/trn_rl_repo"):
    if p not in sys.path:
        sys.path.insert(0, p)

B, L, D = 2, 2048, 1024
H, HD = 16, 64
NCORES = 8
HG = 4                  # head-groups == cores per batch
EL = D // HG            # 256 projection columns per core
ET = EL // 128          # 2 e-tiles
DT = D // 128           # 8 d-tiles
TT = L // 128           # 16 token tiles
NH = H // HG            # 4 heads per core

_cache = {}

LDW_OPT = False  # let walrus dedupe back-to-back identical weight loads


def _patch_ldw_opt():
    import concourse.bass_utils as bu
    if getattr(bu, "_ldw_patched", False):
        return
    orig = bu.run_command

    def patched(argv, **kw):
        argv = ["--enable-ldw-opt=true" if a == "--enable-ldw-opt=false" else a
                for a in argv]
        return orig(argv, **kw)

    bu.run_command = patched
    bu._ldw_patched = True


def _build():
    import concourse.mybir as mybir
    from concourse import bacc, tile

    F32 = mybir.dt.float32
    F32R = mybir.dt.float32r
    AF = mybir.ActivationFunctionType

    nc = bacc.Bacc("TRN2", target_bir_lowering=False, debug=False,
                   num_devices=NCORES)

    xT = nc.dram_tensor("xT", [D, L], F32R, kind="ExternalInput").ap()
    wq = nc.dram_tensor("wq", [D, EL], F32R, kind="ExternalInput").ap()
    wk = nc.dram_tensor("wk", [D, EL], F32R, kind="ExternalInput").ap()
    wv = nc.dram_tensor("wv", [D, EL], F32R, kind="ExternalInput").ap()
    wo = nc.dram_tensor("wo", [EL, D], F32R, kind="ExternalInput").ap()
    bq = nc.dram_tensor("bq", [ET, 128, 1], F32, kind="ExternalInput").ap()
    bk = nc.dram_tensor("bk", [ET, 128, 1], F32, kind="ExternalInput").ap()
    cosb = nc.dram_tensor("cosb", [128, L], F32, kind="ExternalInput").ap()
    onesd = nc.dram_tensor("onesd", [128, NH, 1], F32R, kind="ExternalInput").ap()
    sinb = nc.dram_tensor("sinb", [128, L], F32, kind="ExternalInput").ap()
    outT = nc.dram_tensor("outT", [D, L], F32, kind="ExternalOutput").ap()

    with tile.TileContext(nc) as tc:
        with tc.tile_pool(name="persist", bufs=1) as P:
            qT = [P.tile([128, L], F32R, name=f"qT{e}") for e in range(ET)]
            kT = [P.tile([128, L], F32R, name=f"kT{e}") for e in range(ET)]
            Vsb = [P.tile([128, NH * 65], F32R, name=f"V{t}") for t in range(TT)]
            ao = [P.tile([128, L], F32R, name=f"ao{e}") for e in range(ET)]

            # ---------------- Phase B: q/k/v projections + RoPE ----------
            with (
                tc.tile_pool(name="xw", bufs=1) as XW,
                tc.tile_pool(name="pb", bufs=2, space="PSUM") as PB,
                tc.tile_pool(name="ropet", bufs=2) as RT,
            ):
                xts = [XW.tile([128, L], F32R, name=f"x{d}") for d in range(DT)]
                wqs = [XW.tile([128, EL], F32R, name=f"wq{d}") for d in range(DT)]
                wks = [XW.tile([128, EL], F32R, name=f"wk{d}") for d in range(DT)]
                wvs = [XW.tile([128, EL], F32R, name=f"wv{d}") for d in range(DT)]
                for d in range(DT):
                    nc.sync.dma_start(wqs[d][:], wq[d * 128:(d + 1) * 128, :])
                    nc.sync.dma_start(xts[d][:], xT[d * 128:(d + 1) * 128, :])
                    nc.sync.dma_start(wks[d][:], wk[d * 128:(d + 1) * 128, :])
                    nc.sync.dma_start(wvs[d][:], wv[d * 128:(d + 1) * 128, :])
                cosbt = XW.tile([128, L], F32)
                sinbt = XW.tile([128, L], F32)
                nc.sync.dma_start(cosbt[:], cosb[:])
                nc.sync.dma_start(sinbt[:], sinb[:])
                bqt = [XW.tile([128, 1], F32, name=f"bq{e}") for e in range(ET)]
                bkt = [XW.tile([128, 1], F32, name=f"bk{e}") for e in range(ET)]
                for e in range(ET):
                    nc.sync.dma_start(bqt[e][:], bq[e])
                    nc.sync.dma_start(bkt[e][:], bk[e])

                for wts, bts, dst in ((wqs, bqt, qT), (wks, bkt, kT)):
                    for e in range(ET):
                        ps = PB.tile([128, L], F32, tag="ps")
                        for d in range(DT):
                            for c in range(0, L, 512):
                                nc.tensor.matmul(
                                    ps[:, c:c + 512],
                                    wts[d][:, e * 128:(e + 1) * 128],
                                    xts[d][:, c:c + 512],
                                    start=(d == 0), stop=(d == DT - 1),
                                    skip_group_check=True)
                        nc.vector.tensor_scalar_add(dst[e][:], ps[:], bts[e][:])
                        # RoPE: build rotate_half source via partition-shifted
                        # SBUF->SBUF DMA, then 2 muls + add.
                        rs = RT.tile([128, L], F32, tag="rs")
                        tmp = RT.tile([128, L], F32, tag="tmp")
                        for g in range(4):
                            s0 = g * 32
                            d0 = s0 + 32 if g % 2 == 0 else s0 - 32
                            nc.sync.dma_start(rs[s0:s0 + 32, :],
                                              dst[e][d0:d0 + 32, :].bitcast(F32))
                        nc.vector.tensor_mul(tmp[:], dst[e][:], cosbt[:])
                        nc.vector.tensor_mul(rs[:], rs[:], sinbt[:])
                        nc.vector.tensor_add(dst[e][:], tmp[:], rs[:])

                for t in range(TT):
                    ps = PB.tile([128, EL], F32, tag="ps")
                    for d in range(DT):
                        nc.tensor.matmul(
                            ps[:], xts[d][:, t * 128:(t + 1) * 128],
                            wvs[d][:],
                            start=(d == 0), stop=(d == DT - 1),
                            skip_group_check=True)
                    dv = Vsb[t][:].rearrange("p (h c) -> p h c", c=65)
                    nc.vector.tensor_copy(
                        dv[:, :, 0:64],
                        ps[:].rearrange("p (h c) -> p h c", c=64))
                    nc.sync.dma_start(dv[:, :, 64:65], onesd[:])

            # ---------------- Phase C: attention per head ----------------
            with (
                tc.tile_pool(name="po", bufs=1, space="PSUM") as PO,
                tc.tile_pool(name="pscr", bufs=2, space="PSUM") as PS2,
                tc.tile_pool(name="esb", bufs=3) as EP,
                tc.tile_pool(name="smallsb", bufs=2) as SS,
            ):
                for h in range(NH):
                    e, off = divmod(h, 2)
                    off *= 64
                    qh = qT[e][off:off + 64, :]
                    kh = kT[e][off:off + 64, :]
                    op = PO.tile([65, L], F32, tag="op")
                    for tk in range(TT):
                        # all S chunks for this k-tile first (shared kh
                        # weights back-to-back), then all PV chunks (shared
                        # V' weights back-to-back).
                        ebs = []
                        for c0 in range(0, L, 1024):
                            sp = PS2.tile([128, 1024], F32, tag="scr")
                            for c in range(0, 1024, 512):
                                nc.tensor.matmul(
                                    sp[:, c:c + 512],
                                    kh[:, tk * 128:(tk + 1) * 128],
                                    qh[:, c0 + c:c0 + c + 512],
                                    start=True, stop=True,
                                    skip_group_check=True)
                            eb = EP.tile([128, 1024], F32R, tag="eb")
                            nc.scalar.activation(eb[:], sp[:], AF.Exp,
                                                 scale=0.125)
                            ebs.append(eb)
                        for i, c0 in enumerate(range(0, L, 1024)):
                            for c in range(0, 1024, 512):
                                nc.tensor.matmul(
                                    op[:, c0 + c:c0 + c + 512],
                                    Vsb[tk][:, h * 65:h * 65 + 65],
                                    ebs[i][:, c:c + 512],
                                    start=(tk == 0), stop=(tk == TT - 1),
                                    skip_group_check=True)
                    # Move raw output+denominators to SBUF immediately so the
                    # PSUM accumulator frees for the next head while the
                    # normalize (slow 1-lane reciprocal) runs on DVE fully
                    # overlapped with the next head's matmuls.
                    oraw = SS.tile([65, L], F32, tag="oraw")
                    nc.vector.tensor_copy(oraw[:], op[:])
                    rb = SS.tile([1, L], F32, tag="rb")
                    nc.vector.reciprocal(rb[:], oraw[64:65, :])
                    rbB = SS.tile([64, L], F32, tag="rbB")
                    nc.gpsimd.partition_broadcast(rbB[:], rb[:], channels=64)
                    nc.vector.tensor_mul(ao[e][off:off + 64, :],
                                         oraw[0:64, :], rbB[:])

            # ---------------- Phase D: partial out-projection ------------
            with (
                tc.tile_pool(name="wod", bufs=1) as WOP,
                tc.tile_pool(name="od", bufs=2) as OD,
                tc.tile_pool(name="pd", bufs=2, space="PSUM") as PD,
            ):
                wos = [WOP.tile([128, D], F32R, name=f"wo{e}") for e in range(ET)]
                for e in range(ET):
                    nc.sync.dma_start(wos[e][:], wo[e * 128:(e + 1) * 128, :])
                for dc in range(DT):
                    pdt = PD.tile([128, L], F32, tag="pd")
                    for e in range(ET):
                        for c in range(0, L, 512):
                            nc.tensor.matmul(
                                pdt[:, c:c + 512],
                                wos[e][:, dc * 128:(dc + 1) * 128],
                                ao[e][:, c:c + 512],
                                start=(e == 0), stop=(e == ET - 1),
                                skip_group_check=True)
                    osb = OD.tile([128, L], F32, tag="osb")
                    nc.vector.tensor_copy(osb[:], pdt[:])
                    nc.sync.dma_start(outT[dc * 128:(dc + 1) * 128, :], osb[:])

    nc.compile()
    return nc


def _rope_tables():
    inv = 1.0 / (10000.0 ** (np.arange(0, HD, 2, dtype=np.float32) / HD))
    t = np.arange(L, dtype=np.float32)
    fr = t[:, None] * inv[None, :]                    # [L, 32]
    emb = np.concatenate([fr, fr], axis=1)            # [L, 64]
    cos, sin = np.cos(emb), np.sin(emb)               # [L, 64]
    # device layout [128, L]: row p covers head-dim i = p % 64, two heads
    # stacked per 128-partition tile; sin carries the rotate_half sign.
    i = np.arange(128) % HD
    cosb = cos.T[i, :]                                # [128, L]
    sg = np.where(i < HD // 2, -1.0, 1.0).astype(np.float32)
    sinb = sin.T[i, :] * sg[:, None]
    return np.ascontiguousarray(cosb, np.float32), \
        np.ascontiguousarray(sinb, np.float32)


def _in_maps(x, q_w, q_b, k_w, k_b, v_w, o_w):
    cosb, sinb = _rope_tables()
    qwT = np.ascontiguousarray(np.asarray(q_w, np.float32).T)  # [D, D] eff
    kwT = np.ascontiguousarray(np.asarray(k_w, np.float32).T)
    vwT = np.ascontiguousarray(np.asarray(v_w, np.float32).T)
    owT = np.ascontiguousarray(np.asarray(o_w, np.float32).T)
    xTb = [np.ascontiguousarray(x[b].T) for b in range(B)]
    maps = []
    for c in range(NCORES):
        b, hg = divmod(c, HG)
        er = slice(hg * EL, (hg + 1) * EL)
        maps.append({
            "xT": xTb[b],
            "wq": np.ascontiguousarray(qwT[:, er]),
            "wk": np.ascontiguousarray(kwT[:, er]),
            "wv": np.ascontiguousarray(vwT[:, er]),
            "wo": np.ascontiguousarray(owT[er, :]),
            "bq": np.ascontiguousarray(
                np.asarray(q_b, np.float32)[er].reshape(ET, 128, 1)),
            "bk": np.ascontiguousarray(
                np.asarray(k_b, np.float32)[er].reshape(ET, 128, 1)),
            "cosb": cosb,
            "sinb": sinb,
            "onesd": np.ones((128, NH, 1), np.float32),
        })
    return maps


def kernel(x, q_w, q_b, k_w, k_b, v_w, v_b, o_w, o_b):
    from concourse.bass_utils import run_bass_kernel_spmd

    x = np.asarray(x, np.float32)
    assert x.shape == (B, L, D), x.shape

    if LDW_OPT:
        _patch_ldw_opt()
    if "nc" not in _cache:
        _cache["nc"] = _build()
    nc = _cache["nc"]

    in_maps = _in_maps(x, q_w, q_b, k_w, k_b, v_w, o_w)
    res = run_bass_kernel_spmd(nc, in_maps, list(range(NCORES)))

    out = np.zeros((B, L, D), np.float32)
    for c in range(NCORES):
        b = c // HG
        out[b] += res.results[c]["outT"].T
    # o_b, plus v_b's contribution (v_b flows through softmax-weighted
    # averaging unchanged, then through the out-projection).
    extra = np.asarray(o_b, np.float32) + \
        np.asarray(v_b, np.float32) @ np.asarray(o_w, np.float32).T
    out += extra[None, None, :]
    return out
